# revision 1
# baseline (speedup 1.0000x reference)
"""VQ codebook quantizer (nn_JSCCQ_31550829757033) on 8 NeuronCores.

Data-parallel: the flattened [N, 2] points are sharded across the 8 cores
(pmap); the tiny [64, 2] codebook is replicated. Each core computes squared
distances via a matmul, a numerically-stable softmax, per-shard argmin
indices and per-shard likelihood partial sums. Host side: gather codewords
for the straight-through output and average the likelihood partials
(the only cross-core reduction, per the sharding hint).
"""

import numpy as np
import jax
import jax.numpy as jnp
from functools import partial

SIGMA = 10.0
N_CORES = 8
B, C, H, W = 64, 32, 32, 32          # x shape
N_PTS = B * C * H * W // 2           # 1048576 flattened 2-d points
N_EMBED = 64


@partial(jax.pmap, axis_name="cores")
def _shard_kernel(pts, embed):
    """pts: [n_local, 2] f32, embed: [64, 2] f32 (replicated).

    Returns (embed_ind [n_local] int32, lik_partial [64] f32 — sum of
    softmax rows over the local shard, not yet divided by N).
    """
    # squared euclidean distance, same algebraic form as the reference
    p2 = jnp.sum(pts * pts, axis=1, keepdims=True)            # [n, 1]
    e2 = jnp.sum(embed * embed, axis=1)                       # [64]
    dist = p2 - 2.0 * (pts @ embed.T) + e2                    # [n, 64]
    z = -SIGMA * dist
    zmax = jnp.max(z, axis=1, keepdims=True)
    ez = jnp.exp(z - zmax)                                    # [n, 64]
    soft = ez / jnp.sum(ez, axis=1, keepdims=True)
    lik_partial = jnp.sum(soft, axis=0)                       # [64]
    ind = jnp.argmax(soft, axis=1).astype(jnp.int32)          # [n]
    return ind, lik_partial


def kernel(x, embed):
    x = np.asarray(x, dtype=np.float32)
    embed_np = np.asarray(embed, dtype=np.float32)

    flat = x.reshape(-1, 2)                                   # [N, 2]
    n = flat.shape[0]
    assert n % N_CORES == 0
    shard = n // N_CORES

    pts_sh = flat.reshape(N_CORES, shard, 2)                  # shard points
    emb_sh = np.broadcast_to(embed_np, (N_CORES, N_EMBED, 2)) # replicate

    ind_sh, lik_sh = _shard_kernel(jnp.asarray(pts_sh), jnp.asarray(emb_sh))
    ind = np.asarray(ind_sh).reshape(-1)                      # [N] gather ids
    lik = np.asarray(lik_sh, dtype=np.float64).sum(0)         # all-reduce

    # host-side unshard + output assembly
    hard_q = embed_np[ind]                                    # [N, 2]
    quantize = hard_q.reshape(x.shape[0], -1, 2)              # [B, L, 2]
    likelihoods = (lik / n).astype(np.float32)                # [64]
    return quantize, likelihoods


# revision 3
# speedup vs baseline: 1.4956x; 1.4956x over previous
"""VQ codebook quantizer (nn_JSCCQ_31550829757033) on 8 NeuronCores.

Data-parallel: the flattened [N, 2] points are sharded across the 8 cores
(pmap); the tiny [64, 2] codebook is replicated. Each core computes squared
distances via a matmul, a numerically-stable softmax, per-shard argmin
indices and per-shard likelihood partial sums. Host side: gather codewords
for the straight-through output and average the likelihood partials
(the only cross-core reduction, per the sharding hint).
"""

import numpy as np
import jax
import jax.numpy as jnp
from functools import partial

SIGMA = 10.0
N_CORES = 8
B, C, H, W = 64, 32, 32, 32          # x shape
N_PTS = B * C * H * W // 2           # 1048576 flattened 2-d points
N_EMBED = 64


@partial(jax.pmap, axis_name="cores")
def _shard_kernel(pts, embed):
    """pts: [n_local, 2] f32, embed: [64, 2] f32 (replicated).

    Returns (embed_ind [n_local] int32, lik_partial [64] f32 — sum of
    softmax rows over the local shard, not yet divided by N).
    """
    # squared euclidean distance, same algebraic form as the reference
    p2 = jnp.sum(pts * pts, axis=1, keepdims=True)            # [n, 1]
    e2 = jnp.sum(embed * embed, axis=1)                       # [64]
    dist = p2 - 2.0 * (pts @ embed.T) + e2                    # [n, 64]
    z = -SIGMA * dist
    zmax = jnp.max(z, axis=1, keepdims=True)
    ez = jnp.exp(z - zmax)                                    # [n, 64]
    soft = ez / jnp.sum(ez, axis=1, keepdims=True)
    lik_partial = jnp.sum(soft, axis=0)                       # [64]
    ind = jnp.argmax(soft, axis=1).astype(jnp.int8)           # [n], 64 ids fit
    return ind, lik_partial


def kernel(x, embed):
    x = np.asarray(x, dtype=np.float32)
    embed_np = np.asarray(embed, dtype=np.float32)

    flat = x.reshape(-1, 2)                                   # [N, 2]
    n = flat.shape[0]
    assert n % N_CORES == 0
    shard = n // N_CORES

    pts_sh = flat.reshape(N_CORES, shard, 2)                  # shard points
    emb_sh = np.broadcast_to(embed_np, (N_CORES, N_EMBED, 2)) # replicate

    ind_sh, lik_sh = _shard_kernel(jnp.asarray(pts_sh), jnp.asarray(emb_sh))
    ind = np.asarray(ind_sh).reshape(-1).astype(np.int64)                      # [N] gather ids
    lik = np.asarray(lik_sh, dtype=np.float64).sum(0)         # all-reduce

    # host-side unshard + output assembly
    hard_q = embed_np[ind]                                    # [N, 2]
    quantize = hard_q.reshape(x.shape[0], -1, 2)              # [B, L, 2]
    likelihoods = (lik / n).astype(np.float32)                # [64]
    return quantize, likelihoods
